# revision 1
# baseline (speedup 1.0000x reference)
"""Trainium2 Bass kernel: multi-head attention (B=4, N=2048, D=768, 12 heads).

Sharding: 8 cores = 4 batches x 2 head-groups (6 heads each).
Each core computes, for its (batch, head-group):
    qT/kT = (W[:,cols].T @ x.T)   in [64*2, N] stacked head pairs
    v     = x @ Wv[:,cols]        in [N, 6*64] (+ ones column per head)
    sT    = k q^T (scaled, exp'd) -> attn^T tiles [keys, queries]
    oT    = [v|1].T @ expT        -> unnormalised output + softmax sums
    yT    = Wp[rows,:].T @ (oT / sums)   partial output [768, N]
Host sums the two partial yT per batch (all-reduce of the row-split Wp
projection) and adds bp.

All matmuls bf16 with fp32 PSUM accumulate; softmax exp on ScalarE with
no max-subtraction (scores are O(1) by construction).
"""

import numpy as np
import ml_dtypes

B, N, DIM = 4, 2048, 768
HEADS, HD = 12, 64
SCALE = HD ** -0.5
NCORES = 8
HLOC = HEADS // 2        # heads per core
PAIRS = HLOC // 2        # head pairs per core
P = 128
QB = 512                 # query block
NQB = N // QB            # 4
KT = N // P              # 16 key tiles
KTG = 3                  # key tiles per exp group (psum tile = KTG banks)
GROUPS = [(0, 2), (2, 3), (5, 3), (8, 3), (11, 3), (14, 2)]
KC = DIM // P            # 6 contraction chunks for projections
VPAD = 72                # padded per-head v row (fp8 DoubleRow needs step%16==0)
AV_FP8 = False           # exp weights + v in fp8e4 with DoubleRow attn@v
# Schraudolph bf16 exp on DVE for a subset of score tiles: balances the exp
# work between ScalarE and VectorE. (group_idx, head_idx) pairs handled by DVE.
DVE_EXP = frozenset()
SCHRAU_A = 128.0 / float(np.log(2.0))      # bf16 exponent scale
SCHRAU_B = 16256.0 - 7.4                   # 127<<7 minus centering constant

_cache = {}
EPOOL_BUFS = 8
NPOOL_BUFS = 6


def _build(dump=False):
    import concourse.bacc as bacc
    import concourse.mybir as mybir
    import concourse.tile as tile
    from concourse._compat import get_trn_type

    fp32 = mybir.dt.float32
    bf16 = mybir.dt.bfloat16
    fp8 = mybir.dt.float8e4
    edt = fp8 if AV_FP8 else bf16      # dtype of exp weights / v
    Exp = mybir.ActivationFunctionType.Exp
    mult = mybir.AluOpType.mult

    nc = bacc.Bacc(
        get_trn_type() or "TRN2",
        target_bir_lowering=False,
        debug=False,
        enable_asserts=False,
        num_devices=NCORES,
    )

    xT = nc.dram_tensor("xT", [DIM, N], bf16, kind="ExternalInput").ap()
    wq = nc.dram_tensor("wq", [DIM, HLOC * HD], bf16, kind="ExternalInput").ap()
    wk = nc.dram_tensor("wk", [DIM, HLOC * HD], bf16, kind="ExternalInput").ap()
    wv = nc.dram_tensor("wv", [DIM, HLOC * HD], bf16, kind="ExternalInput").ap()
    wp = nc.dram_tensor("wp", [HLOC * HD, DIM], bf16, kind="ExternalInput").ap()
    yT = nc.dram_tensor("yT", [DIM, N], fp32, kind="ExternalOutput").ap()
    if dump:
        dbg_qT = nc.dram_tensor("dbg_qT", [PAIRS, P, N], bf16, kind="ExternalOutput").ap()
        dbg_kT = nc.dram_tensor("dbg_kT", [PAIRS, P, N], bf16, kind="ExternalOutput").ap()
        dbg_v = nc.dram_tensor("dbg_v", [P, KT, HLOC, VPAD], edt, kind="ExternalOutput").ap()
        dbg_oT = nc.dram_tensor("dbg_oT", [P, PAIRS, N], bf16, kind="ExternalOutput").ap()
        dbg_e = nc.dram_tensor("dbg_e", [2, P, KTG * QB], edt, kind="ExternalOutput").ap()
        dbg_av = nc.dram_tensor("dbg_av", [2, HD + 1, QB], fp32, kind="ExternalOutput").ap()
        dbg_bc = nc.dram_tensor("dbg_bc", [2, HD, QB], fp32, kind="ExternalOutput").ap()

    with tile.TileContext(nc) as tc:
        with (
            tc.tile_pool(name="const", bufs=1) as cpool,
            tc.tile_pool(name="exp", bufs=EPOOL_BUFS) as epool,
            tc.tile_pool(name="norm", bufs=NPOOL_BUFS) as npool,
            tc.tile_pool(name="qkp", bufs=2, space="PSUM") as qkp,
            tc.tile_pool(name="avp", bufs=2, space="PSUM") as avp,
        ):
            # ---------------- input loads ----------------
            # order: small weights first so the first proj matmuls start early
            wq_sb = cpool.tile([P, KC, HLOC * HD], bf16, name="wq_sb")
            wqr = wq.rearrange("(o p) m -> p o m", p=P)
            nc.sync.dma_start(wq_sb[:, :, 0:P], wqr[:, :, 0:P])
            xT_sb = cpool.tile([P, KC, N], bf16, name="xT_sb")
            xTr = xT.rearrange("(o p) n -> p o n", p=P)
            wk_sb = cpool.tile([P, KC, HLOC * HD], bf16, name="wk_sb")
            for t in range(NQB):
                ts_ = slice(t * QB, (t + 1) * QB)
                if t == 0:
                    nc.sync.dma_start(xT_sb[:, 0:3, ts_], xTr[:, 0:3, ts_])
                    nc.sync.dma_start(xT_sb[:, 3:6, ts_], xTr[:, 3:6, ts_])
                else:
                    nc.sync.dma_start(xT_sb[:, :, ts_], xTr[:, :, ts_])
                if t == 1:
                    nc.sync.dma_start(wk_sb[:, :, 0:P],
                                      wk.rearrange("(o p) m -> p o m", p=P)[:, :, 0:P])
            nc.sync.dma_start(wq_sb[:, :, P:], wqr[:, :, P:])
            nc.sync.dma_start(wk_sb[:, :, P:],
                              wk.rearrange("(o p) m -> p o m", p=P)[:, :, P:])
            wv_sb = cpool.tile([P, KC, HLOC * HD], bf16, name="wv_sb")
            nc.sync.dma_start(wv_sb, wv.rearrange("(o p) m -> p o m", p=P))
            wp_sb = cpool.tile([P, PAIRS, DIM], bf16, name="wp_sb")
            nc.sync.dma_start(wp_sb, wp.rearrange("(o p) m -> p o m", p=P))

            # HAM warm-up: dummy matmuls fill the startup DMA wait so the
            # PE clock-gate is already released when the projections start
            warm = cpool.tile([P, QB], bf16, name="warm")
            nc.vector.memset(warm, 0.0)
            wps = qkp.tile([P, KTG * QB], fp32, name="wps", tag="qk")
            for _w in range(8):
                nc.tensor.matmul(wps[:, 0:QB], lhsT=warm[:, 0:P], rhs=warm)

            qT_sb = [cpool.tile([P, N], bf16, name=f"qT{pr}") for pr in range(PAIRS)]
            kT_sb = [cpool.tile([P, N], bf16, name=f"kT{pr}") for pr in range(PAIRS)]
            # v with a trailing ones column per head: [P, kt, head, 64+1]
            v_sb = cpool.tile([P, KT, HLOC, VPAD], edt, name="v_sb")
            oT_sb = cpool.tile([P, PAIRS, N], bf16, name="oT_sb")
            ones64 = cpool.tile([HD + 1, HD], fp32, name="ones64")
            nc.vector.memset(ones64[HD:HD + 1, :], 1.0)
            ebias = cpool.tile([P, 1], fp32, name="ebias")
            nc.vector.memset(ebias, -2.0 if AV_FP8 else 0.0)
            nc.vector.memset(v_sb[:, :, :, HD], 1.0)
            if VPAD > HD + 1:
                # zero the padding so DoubleRow's 16B-granule weight loads
                # never pick up garbage (NaN-encoded) bytes
                nc.vector.memset(v_sb[:, :, :, HD + 1:VPAD], 0.0)

            # ---------------- emission helpers ----------------
            def emit_projqk_group(pair, wi, half):
                    w_sb, dst = ((wq_sb, qT_sb[pair]), (wk_sb, kT_sb[pair]))[wi]
                    ps = qkp.tile([P, KTG * QB], fp32, name="ps_qk", tag="qk")
                    for nb in range(2):
                        col = half * 2 * QB + nb * QB
                        for kc in range(KC):
                            nc.tensor.matmul(
                                ps[:, nb * QB:(nb + 1) * QB],
                                lhsT=w_sb[:, kc, pair * P:(pair + 1) * P],
                                rhs=xT_sb[:, kc, col:col + QB],
                                start=(kc == 0),
                                stop=(kc == KC - 1),
                            )
                    nc.vector.tensor_copy(
                        out=dst[:, half * 2 * QB:(half + 1) * 2 * QB],
                        in_=ps[:, :2 * QB],
                    )

            def emit_projqk(pair, order=((0, 0), (0, 1), (1, 0), (1, 1))):
                # order = sequence of (which-of-q/k, token-half); for pair0
                # q-half0 and k-half0 come first so qb0 attention starts early
                for wi, half in order:
                    emit_projqk_group(pair, wi, half)

            def emit_projv_group(g):
                    ps = qkp.tile([P, KTG * QB], fp32, name="ps_v", tag="qk")
                    for j in range(2):
                        nt = g * 2 + j
                        for kc in range(KC):
                            nc.tensor.matmul(
                                ps[:, j * QB: j * QB + HLOC * HD],
                                lhsT=xT_sb[:, kc, nt * P:(nt + 1) * P],
                                rhs=wv_sb[:, kc, :],
                                start=(kc == 0),
                                stop=(kc == KC - 1),
                            )
                        nc.vector.tensor_copy(
                            out=v_sb[:, nt, :, 0:HD],
                            in_=ps[:, j * QB: j * QB + HLOC * HD].rearrange(
                                "p (h d) -> p h d", d=HD
                            ),
                        )

            def emit_qk_exp(qb, pair, interleave=None):
                qs = slice(qb * QB, (qb + 1) * QB)
                etiles = []
                for g0, glen in GROUPS:
                    psA = qkp.tile([P, KTG * QB], fp32, name="psA", tag="qk")[:, :glen * QB]
                    psB = qkp.tile([P, KTG * QB], fp32, name="psB", tag="qk")[:, :glen * QB]
                    for j in range(glen):
                        kt = g0 + j
                        ks = slice(kt * P, (kt + 1) * P)
                        # sT[keys, queries] for the two heads of the pair,
                        # run concurrently in the two 64-row halves of PE
                        nc.tensor.matmul(
                            psA[:, j * QB:(j + 1) * QB],
                            lhsT=kT_sb[pair][0:HD, ks],
                            rhs=qT_sb[pair][0:HD, qs],
                            tile_position=(0, 0),
                        )
                        nc.tensor.matmul(
                            psB[:, j * QB:(j + 1) * QB],
                            lhsT=kT_sb[pair][HD:P, ks],
                            rhs=qT_sb[pair][HD:P, qs],
                            tile_position=(HD, 0),
                        )
                    eA = epool.tile([P, KTG * QB], edt, name="eA", tag="eA")[:, :glen * QB]
                    eB = epool.tile([P, KTG * QB], edt, name="eB", tag="eB")[:, :glen * QB]
                    gi = len(etiles)
                    for idx, (e, ps) in enumerate(((eA, psA), (eB, psB))):
                        if not AV_FP8 and (gi, idx) in DVE_EXP:
                            # Schraudolph: bf16 bits of exp(s*SCALE) as a
                            # single fused multiply-add + round-to-int16
                            nc.vector.tensor_scalar(
                                e.bitcast(mybir.dt.int16), ps,
                                SCALE * SCHRAU_A, SCHRAU_B,
                                mybir.AluOpType.mult, mybir.AluOpType.add,
                            )
                        else:
                            # bias -2 keeps exp weights inside fp8 range when
                            # AV_FP8; the factor cancels in the normalisation
                            nc.scalar.activation(e, ps, Exp, scale=SCALE,
                                                 bias=ebias[:, :])
                    etiles.append((eA, eB))
                    if interleave is not None:
                        interleave(len(etiles) - 1)
                return etiles

            def emit_av_norm(qb, pair, etiles, fast_norm=False):
                qs = slice(qb * QB, (qb + 1) * QB)
                avs = []
                for idx in range(2):
                    av = avp.tile([HD + 1, QB], fp32, name=f"av{idx}", tag="av")
                    avs.append(av)
                    for gi, (g0, glen) in enumerate(GROUPS):
                        e = etiles[gi][idx]
                        for j in range(glen):
                            kt = g0 + j
                            nc.tensor.matmul(
                                av,
                                lhsT=v_sb[:, kt, 2 * pair + idx, 0:HD + 1],
                                rhs=e[:, j * QB:(j + 1) * QB],
                                start=(kt == 0),
                                stop=(kt == KT - 1),
                            )
                avA, avB = avs
                # normalise by the softmax sums (row HD of av) and evacuate.
                # recip/broadcast only work at base partition 0 on hw, so
                # copy the sums row out of PSUM and DMA-hop it to partition 0.
                # In the terminal (fast_norm) iteration the odd head goes
                # first so its oT DMA hop overlaps the even head's chain, and
                # the copies ride the now-idle ScalarE.
                heads = ((1, avB), (0, avA)) if fast_norm else ((0, avA), (1, avB))
                for idx, av in heads:
                    # evacuate the whole av tile first so its PSUM slot frees
                    # early (costs the same as copying just the sums row)
                    avc = npool.tile([HD + 1, QB], fp32, name="avc", tag="avc")
                    if fast_norm:
                        nc.scalar.copy(avc, av)
                    else:
                        nc.vector.tensor_copy(out=avc, in_=av)
                    if fast_norm:
                        # terminal pair: shorter chain via PE ones-broadcast
                        # of the sums row, then reciprocal at base partition 0
                        bcp = avp.tile([HD, QB], fp32, name="bcp", tag="av")
                        nc.tensor.matmul(bcp, lhsT=ones64[HD:HD + 1, :],
                                         rhs=avc[HD:HD + 1, :])
                        bc = npool.tile([HD, QB], fp32, name="bc", tag="bc")
                        nc.vector.reciprocal_approx_fast(out=bc, in_=bcp)
                    else:
                        rec = npool.tile([1, QB], fp32, name="rec", tag="rec")
                        nc.sync.dma_start(rec, avc[HD:HD + 1, :])
                        nc.vector.reciprocal_approx_fast(out=rec, in_=rec)
                        bc = npool.tile([HD, QB], fp32, name="bc", tag="bc")
                        nc.gpsimd.partition_broadcast(bc, rec)
                    if dump and qb == 0 and pair == 0:
                        nc.sync.dma_start(dbg_av[idx], avc)
                        nc.sync.dma_start(dbg_bc[idx], bc)
                    if idx == 0:
                        nc.vector.tensor_tensor(
                            oT_sb[0:HD, pair, qs], avc[0:HD, :], bc, mult
                        )
                    else:
                        tmp = npool.tile([HD, QB], bf16, name="tmp", tag="tmp")
                        nc.vector.tensor_tensor(tmp, avc[0:HD, :], bc, mult)
                        nc.sync.dma_start(oT_sb[HD:P, pair, qs], tmp)

            def emit_outproj(qb, ms=None, alt_evac=False):
                qs = slice(qb * QB, (qb + 1) * QB)
                for m in (range(DIM // P) if ms is None else ms):
                    yps = avp.tile([P, QB], fp32, name="yps", tag="av")
                    for kc in range(PAIRS):
                        nc.tensor.matmul(
                            yps,
                            lhsT=wp_sb[:, kc, m * P:(m + 1) * P],
                            rhs=oT_sb[:, kc, qs],
                            start=(kc == 0),
                            stop=(kc == PAIRS - 1),
                        )
                    ysb = npool.tile([P, QB], fp32, name="ysb", tag="ysb")
                    if alt_evac and m % 2 == 0:
                        nc.scalar.copy(ysb, yps)
                    else:
                        nc.vector.tensor_copy(out=ysb, in_=yps)
                    # terminal block: spread output DMAs over both HWDGE queues
                    dma_eng = nc.scalar if (alt_evac and m % 2 == 1) else nc.sync
                    dma_eng.dma_start(yT[m * P:(m + 1) * P, qs], ysb)

            # ---------------- schedule ----------------
            # qk proj for pair0 first, then attention starts while the
            # remaining projections fill PE slack; outproj lags one qb so its
            # oT dependency chain never stalls PE.
            emit_projqk(0, order=((0, 0), (1, 0), (1, 1), (0, 1)))
            vg = {"i": 0}

            def _interleave_projv(gi):
                # slot projv groups between (0,0)'s score groups so they are
                # not alloc-chained behind all of its exps
                if vg["i"] < KT // 2:
                    emit_projv_group(vg["i"])
                    vg["i"] += 1

            p2 = {"i": 0}
            P2_ORDER = ((0, 0), (1, 0), (1, 1), (0, 1))

            def _interleave_projqk2(gi):
                # same trick for pair2's q/k projection inside (0,1)
                if p2["i"] < len(P2_ORDER):
                    emit_projqk_group(2, *P2_ORDER[p2["i"]])
                    p2["i"] += 1

            # preamble: (0,0) with projv interleaved, then (1,0)'s scores
            # feed ScalarE while projqk(1) runs on the PE underneath
            et00 = emit_qk_exp(0, 0, interleave=_interleave_projv)
            while vg["i"] < KT // 2:
                emit_projv_group(vg["i"])
                vg["i"] += 1
            emit_av_norm(0, 0, et00)
            et10 = emit_qk_exp(1, 0)
            emit_projqk(1)
            emit_av_norm(1, 0, et10)
            done = {(0, 0), (1, 0)}
            for qb in range(NQB):
                for pair in range(PAIRS):
                    if (qb, pair) in done:
                        continue
                    ivl = _interleave_projqk2 if (qb, pair) == (0, 1) else None
                    et = emit_qk_exp(qb, pair, interleave=ivl)
                    if qb == 0 and pair == 1:
                        while p2["i"] < len(P2_ORDER):
                            emit_projqk_group(2, *P2_ORDER[p2["i"]])
                            p2["i"] += 1
                    emit_av_norm(qb, pair, et,
                                 fast_norm=(qb == NQB - 1 and pair == PAIRS - 1))
                    if qb == 1:
                        if pair == 1:
                            emit_outproj(0, ms=(0, 1, 2))
                        elif pair == 2:
                            emit_outproj(0, ms=(3, 4, 5))
                    elif qb > 1:
                        emit_outproj(qb - 1, ms=range(2 * pair, 2 * pair + 2))
                if dump and qb == NQB - 1:
                    nc.sync.dma_start(dbg_oT, oT_sb)
            emit_outproj(NQB - 1, alt_evac=True)

    nc.compile()
    return nc


def _get_nc():
    nc = _cache.get("nc")
    if nc is None:
        nc = _build()
        _cache["nc"] = nc
    return nc


def make_in_maps(x, Wq, Wk, Wv, Wp):
    bf = ml_dtypes.bfloat16
    x = np.asarray(x, np.float32)
    Wq = np.asarray(Wq, np.float32)
    Wk = np.asarray(Wk, np.float32)
    Wv = np.asarray(Wv, np.float32)
    Wp = np.asarray(Wp, np.float32)
    xTs = [np.ascontiguousarray(x[b].T).astype(bf) for b in range(B)]
    in_maps = []
    for c in range(NCORES):
        b, hg = divmod(c, 2)
        cs = slice(hg * HLOC * HD, (hg + 1) * HLOC * HD)
        in_maps.append(
            {
                "xT": xTs[b],
                "wq": np.ascontiguousarray(Wq[:, cs]).astype(bf),
                "wk": np.ascontiguousarray(Wk[:, cs]).astype(bf),
                "wv": np.ascontiguousarray(Wv[:, cs]).astype(bf),
                "wp": np.ascontiguousarray(Wp[cs, :]).astype(bf),
            }
        )
    return in_maps


def assemble(outs, bp):
    bp32 = np.asarray(bp, np.float32)
    y = np.empty((B, N, DIM), np.float32)
    for b in range(B):
        y[b] = (outs[2 * b]["yT"] + outs[2 * b + 1]["yT"]).T + bp32
    return y


def kernel(x, Wq, Wk, Wv, Wp, bp):
    from concourse.bass_utils import run_bass_kernel_spmd

    nc = _get_nc()
    in_maps = make_in_maps(x, Wq, Wk, Wv, Wp)
    res = run_bass_kernel_spmd(nc, in_maps, core_ids=list(range(NCORES)))
    _cache["last_result"] = res
    return assemble(res.results, bp)

